# revision 2
# baseline (speedup 1.0000x reference)
"""Trainium2 Bass kernel for causal self-attention (B=4, T=2048, C=2048, H=16).

Sharding: 8 cores = 4 batches x 2 head-groups (8 heads each).
Each core computes, for its (batch, head-group):
  A) v = x @ Wv + bv                      -> DRAM spill [T, 1024]   (natural)
  B) qkT = (Wqk^T x^T) + b, RoPE          -> DRAM spill [2048, T]   (transposed)
  C) per head: sT[k,q] = kT^T-matmul, exp, causal mask, PV accumulate,
     denominator via ones-matmul on a DVE-accumulated p_sum -> yT resident
  D) partial_out = yT^T @ wp_rows         -> [T, C] partial
Host sums core pairs per batch and adds b_proj.

All matmuls run in float32r PE mode (fp32 data, full-rate at N=512).
"""

import sys

import numpy as np

sys.path.insert(0, "/opt/trn_rl_repo")

import concourse.bass as bass  # noqa: E402
import concourse.mybir as mybir  # noqa: E402
import concourse.tile as tile  # noqa: E402

F32 = mybir.dt.float32
F32R = mybir.dt.float32r
AF = mybir.ActivationFunctionType

B, T, C = 4, 2048, 2048
H, D = 16, 128
HPC = 8            # heads per core
P = 128
NT = 512           # matmul moving free dim
TT = T // NT       # 4 token tiles
CC = C // P        # 16 contraction chunks over C
QK_CHUNKS = 2 * HPC  # 16 feature chunks (q heads 0-7, then k heads 0-7)
ROPE_BASE = 10000.0

_CACHE = {}


def _mm(nc, out, lhsT, rhs, **kw):
    nc.tensor.matmul(out, lhsT.bitcast(F32R), rhs.bitcast(F32R), **kw)


def build_program():
    nc = bass.Bass(name="csa_tp")

    xt = nc.dram_tensor("xt", (C, T), F32, kind="ExternalInput")
    wqk = nc.dram_tensor("wqk", (C, QK_CHUNKS * P), F32, kind="ExternalInput")
    bqk = nc.dram_tensor("bqk", (P, QK_CHUNKS), F32, kind="ExternalInput")
    wv = nc.dram_tensor("wv", (C, HPC * D), F32, kind="ExternalInput")
    bvrep = nc.dram_tensor("bvrep", (P, HPC * D), F32, kind="ExternalInput")
    cs = nc.dram_tensor("cs", (P, T), F32, kind="ExternalInput")
    msk = nc.dram_tensor("msk", (P, 4 * NT), F32, kind="ExternalInput")
    onesm = nc.dram_tensor("onesm", (P, P), F32, kind="ExternalInput")
    wp = nc.dram_tensor("wp", (HPC * D, C), F32, kind="ExternalInput")
    out = nc.dram_tensor("out", (T, C), F32, kind="ExternalOutput")

    qk_spill = nc.dram_tensor("qk_spill", (QK_CHUNKS * P, T), F32, kind="Internal")
    v_spill = nc.dram_tensor("v_spill", (T, HPC * D), F32, kind="Internal")

    with tile.TileContext(nc) as tc:
        with tc.tile_pool(name="persist", bufs=1) as persist:
            ones_t = persist.tile([P, P], F32, tag="ones", name="ones")
            nc.sync.dma_start(ones_t[:], onesm[:])
            bv_t = persist.tile([P, HPC * D], F32, tag="bv", name="bv")
            nc.sync.dma_start(bv_t[:], bvrep[:])
            cs_t = persist.tile([P, T], F32, tag="cs", name="cs")
            nc.sync.dma_start(cs_t[:], cs[:])
            bqk_t = persist.tile([P, QK_CHUNKS], F32, tag="bqk", name="bqk")
            nc.sync.dma_start(bqk_t[:], bqk[:])
            msk_t = persist.tile([P, 4 * NT], F32, tag="msk", name="msk")
            nc.sync.dma_start(msk_t[:], msk[:])

            # ---------------- phase A: V ----------------
            with (
                tc.tile_pool(name="wv_pool", bufs=1) as wv_pool,
                tc.tile_pool(name="xa_pool", bufs=1) as xa_pool,
                tc.tile_pool(name="va_pool", bufs=1) as va_pool,
                tc.tile_pool(name="psum_a", bufs=1, space="PSUM") as psum_a,
            ):
                wv_t = []
                for c in range(CC):
                    w_ = wv_pool.tile([P, HPC * D], F32, tag=f"wv{c}", name=f"wv{c}")
                    nc.sync.dma_start(w_[:], wv[c * P:(c + 1) * P, :])
                    wv_t.append(w_)
                for t in range(TT):
                    xts = []
                    for c in range(CC):
                        x_ = xa_pool.tile([P, NT], F32, tag="xa", bufs=20, name="xa")
                        nc.sync.dma_start(
                            x_[:], xt[c * P:(c + 1) * P, t * NT:(t + 1) * NT]
                        )
                        xts.append(x_)
                    for m in range(4):
                        mtok = 4 * t + m
                        for n in range(2):
                            ps = psum_a.tile([P, NT], F32, tag="psa", bufs=8,
                                             name="psa")
                            for c in range(CC):
                                _mm(nc, ps[:],
                                    xts[c][:, m * P:(m + 1) * P],
                                    wv_t[c][:, n * NT:(n + 1) * NT],
                                    start=(c == 0), stop=(c == CC - 1))
                            vt = va_pool.tile([P, NT], F32, tag="vt", bufs=4,
                                              name="vt")
                            nc.vector.tensor_add(
                                vt[:], ps[:], bv_t[:, n * NT:(n + 1) * NT]
                            )
                            nc.sync.dma_start(
                                v_spill[mtok * P:(mtok + 1) * P,
                                        n * NT:(n + 1) * NT],
                                vt[:],
                            )

            # ---------------- phase B: qkT + RoPE ----------------
            with (
                tc.tile_pool(name="wq_pool", bufs=1) as wq_pool,
                tc.tile_pool(name="xb_pool", bufs=1) as xb_pool,
                tc.tile_pool(name="rp_pool", bufs=1) as rp_pool,
                tc.tile_pool(name="psum_b", bufs=1, space="PSUM") as psum_b,
            ):
                hd = D // 2
                for fg in range(2):
                    wq_t = []
                    for c in range(CC):
                        w_ = wq_pool.tile([P, 8 * P], F32, tag=f"wq{c}",
                                          name=f"wq{c}")
                        nc.sync.dma_start(
                            w_[:],
                            wqk[c * P:(c + 1) * P, fg * 8 * P:(fg + 1) * 8 * P],
                        )
                        wq_t.append(w_)
                    for t in range(TT):
                        sl = slice(t * NT, (t + 1) * NT)
                        xts = []
                        for c in range(CC):
                            x_ = xb_pool.tile([P, NT], F32, tag="xb", bufs=20,
                                              name="xb")
                            nc.sync.dma_start(
                                x_[:], xt[c * P:(c + 1) * P, sl]
                            )
                            xts.append(x_)
                        for f in range(8):
                            feat = fg * 8 + f
                            ps = psum_b.tile([P, NT], F32, tag="psb", bufs=8,
                                             name="psb")
                            for c in range(CC):
                                _mm(nc, ps[:],
                                    wq_t[c][:, f * P:(f + 1) * P],
                                    xts[c][:],
                                    start=(c == 0), stop=(c == CC - 1))
                            raw = rp_pool.tile([P, NT], F32, tag="raw", bufs=3,
                                               name="raw")
                            nc.scalar.activation(
                                raw[:], ps[:], AF.Identity,
                                bias=bqk_t[:, feat:feat + 1],
                            )
                            rop = rp_pool.tile([P, NT], F32, tag="rop", bufs=3,
                                               name="rop")
                            tmp = rp_pool.tile([hd, NT], F32, tag="rt1", bufs=3,
                                               name="rt1")
                            tmp2 = rp_pool.tile([hd, NT], F32, tag="rt2", bufs=3,
                                                name="rt2")
                            nc.vector.tensor_mul(
                                rop[0:hd, :], raw[0:hd, :], cs_t[0:hd, sl])
                            nc.vector.tensor_mul(
                                tmp[:], raw[hd:P, :], cs_t[hd:P, sl])
                            nc.vector.tensor_sub(
                                rop[0:hd, :], rop[0:hd, :], tmp[:])
                            nc.vector.tensor_mul(
                                rop[hd:P, :], raw[hd:P, :], cs_t[0:hd, sl])
                            nc.vector.tensor_mul(
                                tmp2[:], raw[0:hd, :], cs_t[hd:P, sl])
                            nc.vector.tensor_add(
                                rop[hd:P, :], rop[hd:P, :], tmp2[:])
                            nc.sync.dma_start(
                                qk_spill[feat * P:(feat + 1) * P, sl], rop[:]
                            )

            # ---------------- phase C: SDPA ----------------
            with tc.tile_pool(name="y_res", bufs=1) as y_res:
                y_tiles = [
                    y_res.tile([P, T], F32, tag=f"y{h}", name=f"y{h}")
                    for h in range(HPC)
                ]
                with (
                    tc.tile_pool(name="qk_pool", bufs=1) as qk_pool,
                    tc.tile_pool(name="sd_pool", bufs=1) as sd_pool,
                    tc.tile_pool(name="psum_s", bufs=2, space="PSUM") as psum_s,
                    tc.tile_pool(name="psum_y", bufs=2, space="PSUM") as psum_y,
                    tc.tile_pool(name="psum_d", bufs=2, space="PSUM") as psum_d,
                ):
                    for h in range(HPC):
                        qh = qk_pool.tile([P, T], F32, tag="qh", bufs=2, name="qh")
                        nc.sync.dma_start(qh[:], qk_spill[h * P:(h + 1) * P, :])
                        kh = qk_pool.tile([P, T], F32, tag="kh", bufs=2, name="kh")
                        nc.sync.dma_start(
                            kh[:], qk_spill[(HPC + h) * P:(HPC + h + 1) * P, :]
                        )
                        vh = []
                        for j in range(T // P):
                            v_ = qk_pool.tile([P, P], F32, tag="vh", bufs=32,
                                              name="vh")
                            nc.sync.dma_start(
                                v_[:],
                                v_spill[j * P:(j + 1) * P, h * D:(h + 1) * D],
                            )
                            vh.append(v_)
                        for t in range(TT):
                            qsl = slice(t * NT, (t + 1) * NT)
                            nch = 4 * t + 4
                            psy = psum_y.tile([P, NT], F32, tag="psy", name="psy")
                            p_sum = sd_pool.tile([P, NT], F32, tag="p_sum",
                                                 bufs=2, name="p_sum")
                            for j in range(nch):
                                pss = psum_s.tile([P, NT], F32, tag="pss",
                                                  name="pss")
                                _mm(nc, pss[:],
                                    kh[:, j * P:(j + 1) * P], qh[:, qsl],
                                    start=True, stop=True)
                                p = sd_pool.tile([P, NT], F32, tag="p", bufs=3,
                                                 name="p")
                                nc.scalar.activation(p[:], pss[:], AF.Exp)
                                dpat = j - 4 * t
                                if dpat >= 0:
                                    nc.vector.tensor_mul(
                                        p[:], p[:],
                                        msk_t[:, dpat * NT:(dpat + 1) * NT],
                                    )
                                if j == 0:
                                    nc.vector.tensor_copy(p_sum[:], p[:])
                                else:
                                    nc.vector.tensor_add(p_sum[:], p_sum[:], p[:])
                                _mm(nc, psy[:], vh[j][:], p[:],
                                    start=(j == 0), stop=(j == nch - 1))
                            psd = psum_d.tile([P, NT], F32, tag="psd", name="psd")
                            _mm(nc, psd[:], ones_t[:], p_sum[:],
                                start=True, stop=True)
                            rden = sd_pool.tile([P, NT], F32, tag="rden", bufs=2,
                                                name="rden")
                            nc.vector.reciprocal(rden[:], psd[:])
                            nc.vector.tensor_mul(
                                y_tiles[h][:, qsl], psy[:], rden[:]
                            )

                # ---------------- phase D: projection ----------------
                with (
                    tc.tile_pool(name="wp_pool", bufs=1) as wp_pool,
                    tc.tile_pool(name="ot_pool", bufs=1) as ot_pool,
                    tc.tile_pool(name="psum_o", bufs=1, space="PSUM") as psum_o,
                ):
                    wp_t = []
                    for hh in range(HPC):
                        w_ = wp_pool.tile([P, C], F32, tag=f"wp{hh}",
                                          name=f"wp{hh}")
                        nc.sync.dma_start(w_[:], wp[hh * P:(hh + 1) * P, :])
                        wp_t.append(w_)
                    for m in range(T // P):
                        for n in range(C // NT):
                            ps = psum_o.tile([P, NT], F32, tag="pso", bufs=8,
                                             name="pso")
                            for hh in range(HPC):
                                _mm(nc, ps[:],
                                    y_tiles[hh][:, m * P:(m + 1) * P],
                                    wp_t[hh][:, n * NT:(n + 1) * NT],
                                    start=(hh == 0), stop=(hh == HPC - 1))
                            ot = ot_pool.tile([P, NT], F32, tag="ot", bufs=4,
                                              name="ot")
                            nc.scalar.copy(ot[:], ps[:])
                            nc.sync.dma_start(
                                out[m * P:(m + 1) * P, n * NT:(n + 1) * NT],
                                ot[:],
                            )

    nc.finalize()
    return nc


def prep_inputs(x, w_attn, b_attn, w_proj, b_proj):
    """Build the 8 per-core input maps from full inputs."""
    x = np.asarray(x, dtype=np.float32)
    w_attn = np.asarray(w_attn, dtype=np.float32)
    b_attn = np.asarray(b_attn, dtype=np.float32)
    w_proj = np.asarray(w_proj, dtype=np.float32)

    scale = np.float32(1.0 / np.sqrt(D))

    inv_freq = 1.0 / (ROPE_BASE ** (np.arange(0, D, 2, dtype=np.float32) / D))
    tpos = np.arange(T, dtype=np.float32)
    ang = np.outer(tpos, inv_freq)  # [T, 64]
    cs = np.ascontiguousarray(
        np.concatenate([np.cos(ang).T, np.sin(ang).T], axis=0)
    ).astype(np.float32)  # [128, T]

    qq = np.arange(NT)
    kk = np.arange(P)[:, None]
    msk = np.ascontiguousarray(np.concatenate(
        [(qq[None, :] >= kk + 128 * dd).astype(np.float32) for dd in range(4)],
        axis=1,
    ))  # [128, 4*NT]

    onesm = np.ones((P, P), dtype=np.float32)

    in_maps = []
    for core in range(8):
        b = core // 2
        hg = core % 2
        heads = range(hg * HPC, (hg + 1) * HPC)
        qcols = np.concatenate([np.arange(h * D, (h + 1) * D) for h in heads])
        kcols = qcols + C
        vcols = qcols + 2 * C

        wq = w_attn[:, qcols] * scale
        wk = w_attn[:, kcols]
        wqk_s = np.ascontiguousarray(np.concatenate([wq, wk], axis=1))
        bqk_s = np.concatenate([b_attn[qcols] * scale, b_attn[kcols]])
        # [128, 16]: column f = per-partition bias of feature chunk f
        bqk_s = np.ascontiguousarray(bqk_s.reshape(QK_CHUNKS, P).T)
        wv_s = np.ascontiguousarray(w_attn[:, vcols])
        bv_s = np.ascontiguousarray(
            np.broadcast_to(b_attn[vcols][None, :], (P, HPC * D))
        )
        wp_s = np.ascontiguousarray(w_proj[qcols, :])
        xt_s = np.ascontiguousarray(x[b].T)

        in_maps.append({
            "xt": xt_s, "wqk": wqk_s, "bqk": bqk_s, "wv": wv_s, "bvrep": bv_s,
            "cs": cs, "msk": msk, "onesm": onesm, "wp": wp_s,
        })
    return in_maps


def _get_program():
    if "nc" not in _CACHE:
        _CACHE["nc"] = build_program()
    return _CACHE["nc"]


def _postprocess(outs, b_proj):
    b_proj = np.asarray(b_proj, dtype=np.float32)
    return np.stack(
        [outs[2 * b] + outs[2 * b + 1] + b_proj[None, :] for b in range(B)]
    ).astype(np.float32)


def _run(inputs, trace=False):
    from concourse.bass_utils import run_bass_kernel_spmd

    nc = _get_program()
    in_maps = prep_inputs(
        inputs["x"], inputs["w_attn"], inputs["b_attn"],
        inputs["w_proj"], inputs["b_proj"],
    )
    res = run_bass_kernel_spmd(nc, in_maps, core_ids=list(range(8)), trace=trace)
    full = _postprocess([r["out"] for r in res.results], inputs["b_proj"])
    return full, res


def kernel(**inputs):
    full, _ = _run(inputs, trace=False)
    return full


if __name__ == "__main__":
    _get_program()
    print("built ok")


# revision 5
# speedup vs baseline: 1.1950x; 1.1950x over previous
"""Trainium2 Bass kernel for causal self-attention (B=4, T=2048, C=2048, H=16).

Sharding: 8 cores = 4 batches x 2 head-groups (8 heads each).
Per core (its batch b, its 8 heads):
  A) v = x @ Wv + bv                     -> DRAM spill [T, 1024]   (natural)
  B) qkT = (Wqk^T x^T) + b, RoPE         -> DRAM spill [2048, T]   (transposed)
  C) per head: sT[k,q] matmuls, exp, causal mask, PV accumulate, denominator
     via ones-matmul on DVE-accumulated p_sum -> yT resident
  D) partial_out = yT^T @ wp_rows        -> [T, C] partial
Host sums core pairs per batch and adds b_proj.

All matmuls in float32r (fp32 data, full PE rate at N>=256). x^T stays
resident in SBUF across phases A+B as 64 [128,512] tiles.
"""

import sys

import numpy as np

sys.path.insert(0, "/opt/trn_rl_repo")

import concourse.bass as bass  # noqa: E402,F401
import concourse.mybir as mybir  # noqa: E402
import concourse.tile as tile  # noqa: E402
from concourse import bacc  # noqa: E402

F32 = mybir.dt.float32
F32R = mybir.dt.float32r
AF = mybir.ActivationFunctionType

B, T, C = 4, 2048, 2048
H, D = 16, 128
HPC = 8            # heads per core
P = 128
NT = 512           # matmul moving free dim
TT = T // NT       # 4 token tiles
CC = C // P        # 16 contraction chunks over C
QK_CHUNKS = 2 * HPC  # 16 feature chunks (q heads 0-7, then k heads 0-7)
ROPE_BASE = 10000.0

_CACHE = {}


def _mm(nc, out, lhsT, rhs, **kw):
    nc.tensor.matmul(out, lhsT, rhs, **kw)


def build_program():
    nc = bacc.Bacc(name="csa_tp")

    xt = nc.dram_tensor("xt", (C, T), F32R, kind="ExternalInput")
    wqk = nc.dram_tensor("wqk", (C, QK_CHUNKS * P), F32R, kind="ExternalInput")
    bqk = nc.dram_tensor("bqk", (P, QK_CHUNKS), F32, kind="ExternalInput")
    wv = nc.dram_tensor("wv", (C, HPC * D), F32R, kind="ExternalInput")
    bvrep = nc.dram_tensor("bvrep", (P, HPC * D), F32, kind="ExternalInput")
    cs = nc.dram_tensor("cs", (P, T), F32, kind="ExternalInput")
    sw = nc.dram_tensor("sw", (P, T), F32, kind="ExternalInput")
    tri = nc.dram_tensor("tri", (P, P), F32, kind="ExternalInput")
    onesm = nc.dram_tensor("onesm", (P, P), F32R, kind="ExternalInput")
    wp = nc.dram_tensor("wp", (HPC * D, C), F32R, kind="ExternalInput")
    out = nc.dram_tensor("out", (T, C), F32, kind="ExternalOutput")

    qk_spill = nc.dram_tensor("qk_spill", (QK_CHUNKS * P, T), F32R,
                              kind="Internal")
    v_spill = nc.dram_tensor("v_spill", (T, HPC * D), F32R, kind="Internal")

    with tile.TileContext(nc) as tc:
        with tc.tile_pool(name="persist", bufs=1) as persist:
            ones_t = persist.tile([P, P], F32R, tag="ones", name="ones")
            nc.sync.dma_start(ones_t[:], onesm[:])
            bqk_t = persist.tile([P, QK_CHUNKS], F32, tag="bqk", name="bqk")
            nc.sync.dma_start(bqk_t[:], bqk[:])

            # x^T resident for phases A+B: 64 tiles [128, 512]
            with tc.tile_pool(name="xt_res", bufs=1) as xt_res:
                # ---------------- phase A: V ----------------
                with (
                    tc.tile_pool(name="wv_pool", bufs=1) as wv_pool,
                    tc.tile_pool(name="va_pool", bufs=1) as va_pool,
                    tc.tile_pool(name="psum_a", bufs=1, space="PSUM") as psum_a,
                ):
                    wv_t = []
                    for c in range(CC):
                        w_ = wv_pool.tile([P, HPC * D], F32R, tag=f"wv{c}",
                                          name=f"wv{c}")
                        nc.sync.dma_start(w_[:], wv[c * P:(c + 1) * P, :])
                        wv_t.append(w_)
                    xtt = [[None] * TT for _ in range(CC)]
                    for t in range(TT):
                        for c in range(CC):
                            x_ = xt_res.tile([P, NT], F32R, tag=f"x{c}_{t}",
                                             name=f"x{c}_{t}")
                            nc.sync.dma_start(
                                x_[:],
                                xt[c * P:(c + 1) * P, t * NT:(t + 1) * NT],
                            )
                            xtt[c][t] = x_
                    bv_t = va_pool.tile([P, HPC * D], F32, tag="bv", name="bv")
                    nc.sync.dma_start(bv_t[:], bvrep[:])

                    for t in range(TT):
                        for m in range(4):
                            mtok = 4 * t + m
                            msl = slice(m * P, (m + 1) * P)
                            ps0 = psum_a.tile([P, NT], F32, tag="psa0", bufs=4,
                                              name="psa0")
                            ps1 = psum_a.tile([P, NT], F32, tag="psa1", bufs=4,
                                              name="psa1")
                            for c in range(CC):
                                lhsT = xtt[c][t][:, msl]
                                _mm(nc, ps0[:], lhsT, wv_t[c][:, 0:NT],
                                    start=(c == 0), stop=(c == CC - 1))
                                _mm(nc, ps1[:], lhsT, wv_t[c][:, NT:2 * NT],
                                    start=(c == 0), stop=(c == CC - 1))
                            for n, psn in ((0, ps0), (1, ps1)):
                                vt = va_pool.tile([P, NT], F32R, tag="vt",
                                                  bufs=4, name="vt")
                                nc.vector.tensor_add(
                                    vt[:], psn[:], bv_t[:, n * NT:(n + 1) * NT]
                                )
                                nc.sync.dma_start(
                                    v_spill[mtok * P:(mtok + 1) * P,
                                            n * NT:(n + 1) * NT],
                                    vt[:],
                                )

                # ---------------- phase B: qkT + RoPE ----------------
                # weight-stationary, f-outer: each lhsT serves 4 matmuls
                with (
                    tc.tile_pool(name="wq_pool", bufs=1) as wq_pool,
                    tc.tile_pool(name="rp_pool", bufs=1) as rp_pool,
                    tc.tile_pool(name="psum_b", bufs=1, space="PSUM") as psum_b,
                ):
                    cs_t = rp_pool.tile([P, T], F32, tag="cs", name="cs")
                    nc.sync.dma_start(cs_t[:], cs[:])
                    sw_t = rp_pool.tile([P, T], F32, tag="sw", name="sw")
                    nc.sync.dma_start(sw_t[:], sw[:])
                    hd = D // 2
                    for fg in range(8):  # 2 feature chunks per group
                        wq_t = []
                        for c in range(CC):
                            w_ = wq_pool.tile([P, 2 * P], F32R, tag=f"wq{c}",
                                              bufs=2, name=f"wq{c}")
                            nc.sync.dma_start(
                                w_[:],
                                wqk[c * P:(c + 1) * P,
                                    fg * 2 * P:(fg + 1) * 2 * P],
                            )
                            wq_t.append(w_)
                        for f in range(2):
                            feat = fg * 2 + f
                            pst = [
                                psum_b.tile([P, NT], F32, tag=f"psb{t}",
                                            bufs=2, name=f"psb{t}")
                                for t in range(TT)
                            ]
                            for c in range(CC):
                                lhsT = wq_t[c][:, f * P:(f + 1) * P]
                                for t in range(TT):
                                    _mm(nc, pst[t][:], lhsT, xtt[c][t][:],
                                        start=(c == 0), stop=(c == CC - 1))
                            for t in range(TT):
                                sl = slice(t * NT, (t + 1) * NT)
                                ps = pst[t]
                                raw = rp_pool.tile([P, NT], F32, tag="raw",
                                                   bufs=3, name="raw")
                                nc.scalar.activation(
                                    raw[:], ps[:], AF.Identity,
                                    bias=bqk_t[:, feat:feat + 1],
                                )
                                rsw = rp_pool.tile([P, NT], F32, tag="rsw",
                                                   bufs=3, name="rsw")
                                nc.scalar.activation(
                                    rsw[0:hd, :], ps[hd:P, :], AF.Identity,
                                    bias=bqk_t[hd:P, feat:feat + 1],
                                )
                                nc.scalar.activation(
                                    rsw[hd:P, :], ps[0:hd, :], AF.Identity,
                                    bias=bqk_t[0:hd, feat:feat + 1],
                                )
                                rop = rp_pool.tile([P, NT], F32R, tag="rop",
                                                   bufs=3, name="rop")
                                tmp = rp_pool.tile([P, NT], F32, tag="rt1",
                                                   bufs=3, name="rt1")
                                nc.vector.tensor_mul(
                                    rop[:], raw[:], cs_t[:, sl])
                                nc.vector.tensor_mul(
                                    tmp[:], rsw[:], sw_t[:, sl])
                                nc.vector.tensor_add(rop[:], rop[:], tmp[:])
                                nc.sync.dma_start(
                                    qk_spill[feat * P:(feat + 1) * P, sl],
                                    rop[:],
                                )

            # ---------------- phase C: SDPA (j-outer) ----------------
            with tc.tile_pool(name="y_res", bufs=1) as y_res:
                y_tiles = [
                    y_res.tile([P, T], F32R, tag=f"y{h}", name=f"y{h}")
                    for h in range(HPC)
                ]
                with tc.tile_pool(name="wp_pool", bufs=1) as wp_pool:
                  # prefetch projection weights during SDPA
                  wp_t = []
                  for hh in range(HPC):
                      w_ = wp_pool.tile([P, C], F32R, tag=f"wp{hh}",
                                        name=f"wp{hh}")
                      nc.sync.dma_start(w_[:], wp[hh * P:(hh + 1) * P, :])
                      wp_t.append(w_)
                  with (
                    tc.tile_pool(name="qk_pool", bufs=1) as qk_pool,
                    tc.tile_pool(name="sd_pool", bufs=1) as sd_pool,
                    tc.tile_pool(name="psum_c", bufs=1, space="PSUM") as psum_c,
                  ):
                    tri_t = qk_pool.tile([P, P], F32, tag="tri", name="tri")
                    nc.sync.dma_start(tri_t[:], tri[:])

                    for h in range(HPC):
                        qh = qk_pool.tile([P, T], F32R, tag="qh", bufs=2,
                                          name="qh")
                        nc.sync.dma_start(qh[:], qk_spill[h * P:(h + 1) * P, :])
                        kh = qk_pool.tile([P, T], F32R, tag="kh", bufs=2,
                                          name="kh")
                        nc.sync.dma_start(
                            kh[:], qk_spill[(HPC + h) * P:(HPC + h + 1) * P, :]
                        )
                        # v for this head: one DMA, [128, 16*128] (j, d)
                        vh3 = qk_pool.tile([P, T // P, P], F32R, tag="vh",
                                           bufs=2, name="vh3")
                        nc.sync.dma_start(
                            vh3[:],
                            v_spill[:, h * D:(h + 1) * D].rearrange(
                                "(j p) d -> p j d", p=P
                            ),
                        )
                        psy = [
                            psum_c.tile([P, NT], F32, tag=f"psy{t}", bufs=1,
                                        name=f"psy{t}")
                            for t in range(TT)
                        ]
                        p_sum = [
                            sd_pool.tile([P, NT], F32R, tag=f"p_sum{t}",
                                         bufs=2, name=f"p_sum{t}")
                            for t in range(TT)
                        ]
                        for j in range(T // P):
                            jsl = slice(j * P, (j + 1) * P)
                            for t in range(j // 4, TT):
                                diag = (t == j // 4)
                                off = (j % 4) * P if diag else 0
                                qsl = slice(t * NT + off, (t + 1) * NT)
                                pss = psum_c.tile([P, NT], F32, tag="pss",
                                                  bufs=3, name="pss")
                                _mm(nc, pss[:, off:], kh[:, jsl], qh[:, qsl],
                                    start=True, stop=True)
                                p = sd_pool.tile([P, NT], F32R, tag="p",
                                                 bufs=4, name="p")
                                nc.scalar.activation(
                                    p[:, off:], pss[:, off:], AF.Exp)
                                if diag:
                                    nc.vector.tensor_mul(
                                        p[:, off:off + P],
                                        p[:, off:off + P],
                                        tri_t[:],
                                    )
                                if j == 0:
                                    nc.vector.tensor_copy(p_sum[t][:], p[:])
                                else:
                                    nc.vector.tensor_add(
                                        p_sum[t][:, off:], p_sum[t][:, off:],
                                        p[:, off:],
                                    )
                                _mm(nc, psy[t][:, off:], vh3[:, j, :],
                                    p[:, off:],
                                    start=(j == 0), stop=(j == 4 * t + 3))
                        for t in range(TT):
                            psd = psum_c.tile([P, NT], F32, tag="psd", bufs=1,
                                              name="psd")
                            _mm(nc, psd[:], ones_t[:], p_sum[t][:],
                                start=True, stop=True)
                            rden = sd_pool.tile([P, NT], F32, tag="rden",
                                                bufs=2, name="rden")
                            nc.vector.reciprocal(rden[:], psd[:])
                            nc.vector.tensor_mul(
                                y_tiles[h][:, t * NT:(t + 1) * NT],
                                psy[t][:], rden[:],
                            )

                  # ---------------- phase D: projection ----------------
                  with (
                      tc.tile_pool(name="ot_pool", bufs=1) as ot_pool,
                      tc.tile_pool(name="psum_o", bufs=1,
                                   space="PSUM") as psum_o,
                  ):
                      for m in range(T // P):
                          msl = slice(m * P, (m + 1) * P)
                          pso = [
                              psum_o.tile([P, NT], F32, tag=f"pso{n}",
                                          bufs=2, name=f"pso{n}")
                              for n in range(4)
                          ]
                          for hh in range(HPC):
                              lhsT = y_tiles[hh][:, msl]
                              for n in range(4):
                                  _mm(nc, pso[n][:], lhsT,
                                      wp_t[hh][:, n * NT:(n + 1) * NT],
                                      start=(hh == 0), stop=(hh == HPC - 1))
                          ot = ot_pool.tile([P, C], F32, tag="ot", bufs=3,
                                            name="ot")
                          for n in range(4):
                              nc.scalar.copy(
                                  ot[:, n * NT:(n + 1) * NT], pso[n][:])
                          nc.sync.dma_start(out[msl, :], ot[:])

    nc.finalize()
    return nc


def prep_inputs(x, w_attn, b_attn, w_proj, b_proj):
    """Build the 8 per-core input maps from full inputs."""
    x = np.asarray(x, dtype=np.float32)
    w_attn = np.asarray(w_attn, dtype=np.float32)
    b_attn = np.asarray(b_attn, dtype=np.float32)
    w_proj = np.asarray(w_proj, dtype=np.float32)

    scale = np.float32(1.0 / np.sqrt(D))

    inv_freq = 1.0 / (ROPE_BASE ** (np.arange(0, D, 2, dtype=np.float32) / D))
    tpos = np.arange(T, dtype=np.float32)
    ang = np.outer(tpos, inv_freq)  # [T, 64]
    cos_t, sin_t = np.cos(ang).T, np.sin(ang).T  # [64, T]
    cs = np.ascontiguousarray(
        np.concatenate([cos_t, cos_t], axis=0)).astype(np.float32)
    sw = np.ascontiguousarray(
        np.concatenate([-sin_t, sin_t], axis=0)).astype(np.float32)

    qq = np.arange(P)
    kk = np.arange(P)[:, None]
    tri = np.ascontiguousarray(
        (qq[None, :] >= kk).astype(np.float32))  # [128,128] causal triangle

    onesm = np.ones((P, P), dtype=np.float32)

    in_maps = []
    for core in range(8):
        b = core // 2
        hg = core % 2
        heads = range(hg * HPC, (hg + 1) * HPC)
        qcols = np.concatenate([np.arange(h * D, (h + 1) * D) for h in heads])
        kcols = qcols + C
        vcols = qcols + 2 * C

        wq = w_attn[:, qcols] * scale
        wk = w_attn[:, kcols]
        wqk_s = np.ascontiguousarray(np.concatenate([wq, wk], axis=1))
        bqk_s = np.concatenate([b_attn[qcols] * scale, b_attn[kcols]])
        bqk_s = np.ascontiguousarray(bqk_s.reshape(QK_CHUNKS, P).T)
        wv_s = np.ascontiguousarray(w_attn[:, vcols])
        bv_s = np.ascontiguousarray(
            np.broadcast_to(b_attn[vcols][None, :], (P, HPC * D)))
        wp_s = np.ascontiguousarray(w_proj[qcols, :])
        xt_s = np.ascontiguousarray(x[b].T)

        in_maps.append({
            "xt": xt_s, "wqk": wqk_s, "bqk": bqk_s, "wv": wv_s, "bvrep": bv_s,
            "cs": cs, "sw": sw, "tri": tri, "onesm": onesm, "wp": wp_s,
        })
    return in_maps


def _get_program():
    if "nc" not in _CACHE:
        _CACHE["nc"] = build_program()
    return _CACHE["nc"]


def _postprocess(outs, b_proj):
    b_proj = np.asarray(b_proj, dtype=np.float32)
    return np.stack(
        [outs[2 * b] + outs[2 * b + 1] + b_proj[None, :] for b in range(B)]
    ).astype(np.float32)


def _run(inputs, trace=False):
    from concourse.bass_utils import run_bass_kernel_spmd

    nc = _get_program()
    in_maps = prep_inputs(
        inputs["x"], inputs["w_attn"], inputs["b_attn"],
        inputs["w_proj"], inputs["b_proj"],
    )
    res = run_bass_kernel_spmd(nc, in_maps, core_ids=list(range(8)),
                               trace=trace)
    full = _postprocess([r["out"] for r in res.results], inputs["b_proj"])
    return full, res


def kernel(**inputs):
    full, _ = _run(inputs, trace=False)
    return full


if __name__ == "__main__":
    _get_program()
    print("built ok")


# revision 6
# speedup vs baseline: 1.2255x; 1.0255x over previous
"""Trainium2 Bass kernel for causal self-attention (B=4, T=2048, C=2048, H=16).

Sharding: 8 cores = 4 batches x 2 head-groups (8 heads each).
Per core (its batch b, its 8 heads):
  A) v = x @ Wv + bv                     -> DRAM spill [T, 1024]   (natural)
  B) qkT = (Wqk^T x^T) + b, RoPE         -> DRAM spill [2048, T]   (transposed)
  C) per head: sT[k,q] matmuls, exp, causal mask, PV accumulate, denominator
     via ones-matmul on DVE-accumulated p_sum -> yT resident
  D) partial_out = yT^T @ wp_rows        -> [T, C] partial
Host sums core pairs per batch and adds b_proj.

All matmuls in float32r (fp32 data, full PE rate at N>=256). x^T stays
resident in SBUF across phases A+B as 64 [128,512] tiles.
"""

import sys

import numpy as np

sys.path.insert(0, "/opt/trn_rl_repo")

import concourse.bass as bass  # noqa: E402,F401
import concourse.mybir as mybir  # noqa: E402
import concourse.tile as tile  # noqa: E402
from concourse import bacc  # noqa: E402

F32 = mybir.dt.float32
F32R = mybir.dt.float32r
AF = mybir.ActivationFunctionType

B, T, C = 4, 2048, 2048
H, D = 16, 128
HPC = 8            # heads per core
P = 128
NT = 512           # matmul moving free dim
TT = T // NT       # 4 token tiles
CC = C // P        # 16 contraction chunks over C
QK_CHUNKS = 2 * HPC  # 16 feature chunks (q heads 0-7, then k heads 0-7)
ROPE_BASE = 10000.0

_CACHE = {}


def _mm(nc, out, lhsT, rhs, **kw):
    nc.tensor.matmul(out, lhsT, rhs, **kw)


def build_program():
    nc = bacc.Bacc(name="csa_tp")

    xt = nc.dram_tensor("xt", (C, T), F32R, kind="ExternalInput")
    wqk = nc.dram_tensor("wqk", (C, QK_CHUNKS * P), F32R, kind="ExternalInput")
    bqk = nc.dram_tensor("bqk", (P, QK_CHUNKS), F32, kind="ExternalInput")
    wv = nc.dram_tensor("wv", (C, HPC * D), F32R, kind="ExternalInput")
    bvrep = nc.dram_tensor("bvrep", (P, HPC * D), F32, kind="ExternalInput")
    cs = nc.dram_tensor("cs", (P, T), F32, kind="ExternalInput")
    sw = nc.dram_tensor("sw", (P, T), F32, kind="ExternalInput")
    tri = nc.dram_tensor("tri", (P, P), F32, kind="ExternalInput")
    onesm = nc.dram_tensor("onesm", (P, P), F32R, kind="ExternalInput")
    wp = nc.dram_tensor("wp", (HPC * D, C), F32R, kind="ExternalInput")
    out = nc.dram_tensor("out", (T, C), F32, kind="ExternalOutput")

    qk_spill = nc.dram_tensor("qk_spill", (QK_CHUNKS * P, T), F32R,
                              kind="Internal")
    v_spill = nc.dram_tensor("v_spill", (T, HPC * D), F32R, kind="Internal")

    with tile.TileContext(nc) as tc:
        with tc.tile_pool(name="persist", bufs=1) as persist:
            ones_t = persist.tile([P, P], F32R, tag="ones", name="ones")
            nc.sync.dma_start(ones_t[:], onesm[:])
            bqk_t = persist.tile([P, QK_CHUNKS], F32, tag="bqk", name="bqk")
            nc.sync.dma_start(bqk_t[:], bqk[:])

            # x^T resident for phases A+B: 64 tiles [128, 512]
            with tc.tile_pool(name="xt_res", bufs=1) as xt_res:
                # ---------------- phase A: V ----------------
                with (
                    tc.tile_pool(name="wv_pool", bufs=1) as wv_pool,
                    tc.tile_pool(name="va_pool", bufs=1) as va_pool,
                    tc.tile_pool(name="psum_a", bufs=1, space="PSUM") as psum_a,
                ):
                    bv_t = va_pool.tile([P, HPC * D], F32, tag="bv", name="bv")
                    nc.sync.dma_start(bv_t[:], bvrep[:])
                    wv_t = []
                    for c in range(CC):
                        w_ = wv_pool.tile([P, HPC * D], F32R, tag=f"wv{c}",
                                          name=f"wv{c}")
                        nc.sync.dma_start(w_[:], wv[c * P:(c + 1) * P, :])
                        wv_t.append(w_)
                    xtt = [[None] * TT for _ in range(CC)]
                    for t in range(TT):
                        for c in range(CC):
                            x_ = xt_res.tile([P, NT], F32R, tag=f"x{c}_{t}",
                                             name=f"x{c}_{t}")
                            nc.sync.dma_start(
                                x_[:],
                                xt[c * P:(c + 1) * P, t * NT:(t + 1) * NT],
                            )
                            xtt[c][t] = x_
                    for t in range(TT):
                        for m in range(4):
                            mtok = 4 * t + m
                            msl = slice(m * P, (m + 1) * P)
                            ps0 = psum_a.tile([P, NT], F32, tag="psa0", bufs=4,
                                              name="psa0")
                            ps1 = psum_a.tile([P, NT], F32, tag="psa1", bufs=4,
                                              name="psa1")
                            for c in range(CC):
                                lhsT = xtt[c][t][:, msl]
                                _mm(nc, ps0[:], lhsT, wv_t[c][:, 0:NT],
                                    start=(c == 0), stop=(c == CC - 1))
                                _mm(nc, ps1[:], lhsT, wv_t[c][:, NT:2 * NT],
                                    start=(c == 0), stop=(c == CC - 1))
                            for n, psn in ((0, ps0), (1, ps1)):
                                vt = va_pool.tile([P, NT], F32R, tag="vt",
                                                  bufs=4, name="vt")
                                nc.vector.tensor_add(
                                    vt[:], psn[:], bv_t[:, n * NT:(n + 1) * NT]
                                )
                                nc.scalar.dma_start(
                                    v_spill[mtok * P:(mtok + 1) * P,
                                            n * NT:(n + 1) * NT],
                                    vt[:],
                                )

                # ---------------- phase B: qkT + RoPE ----------------
                # weight-stationary, f-outer: each lhsT serves 4 matmuls
                with (
                    tc.tile_pool(name="wq_pool", bufs=1) as wq_pool,
                    tc.tile_pool(name="rp_pool", bufs=1) as rp_pool,
                    tc.tile_pool(name="psum_b", bufs=1, space="PSUM") as psum_b,
                ):
                    cs_t = rp_pool.tile([P, T], F32, tag="cs", name="cs")
                    nc.sync.dma_start(cs_t[:], cs[:])
                    sw_t = rp_pool.tile([P, T], F32, tag="sw", name="sw")
                    nc.sync.dma_start(sw_t[:], sw[:])
                    hd = D // 2
                    for fg in range(8):  # 2 feature chunks per group
                        wq_t = []
                        for c in range(CC):
                            w_ = wq_pool.tile([P, 2 * P], F32R, tag=f"wq{c}",
                                              bufs=2, name=f"wq{c}")
                            nc.sync.dma_start(
                                w_[:],
                                wqk[c * P:(c + 1) * P,
                                    fg * 2 * P:(fg + 1) * 2 * P],
                            )
                            wq_t.append(w_)
                        for f in range(2):
                            feat = fg * 2 + f
                            pst = [
                                psum_b.tile([P, NT], F32, tag=f"psb{t}",
                                            bufs=2, name=f"psb{t}")
                                for t in range(TT)
                            ]
                            for c in range(CC):
                                lhsT = wq_t[c][:, f * P:(f + 1) * P]
                                for t in range(TT):
                                    _mm(nc, pst[t][:], lhsT, xtt[c][t][:],
                                        start=(c == 0), stop=(c == CC - 1))
                            for t in range(TT):
                                sl = slice(t * NT, (t + 1) * NT)
                                ps = pst[t]
                                raw = rp_pool.tile([P, NT], F32, tag="raw",
                                                   bufs=3, name="raw")
                                nc.scalar.activation(
                                    raw[:], ps[:], AF.Identity,
                                    bias=bqk_t[:, feat:feat + 1],
                                )
                                rsw = rp_pool.tile([P, NT], F32, tag="rsw",
                                                   bufs=3, name="rsw")
                                nc.scalar.activation(
                                    rsw[0:hd, :], ps[hd:P, :], AF.Identity,
                                    bias=bqk_t[hd:P, feat:feat + 1],
                                )
                                nc.scalar.activation(
                                    rsw[hd:P, :], ps[0:hd, :], AF.Identity,
                                    bias=bqk_t[0:hd, feat:feat + 1],
                                )
                                rop = rp_pool.tile([P, NT], F32R, tag="rop",
                                                   bufs=3, name="rop")
                                tmp = rp_pool.tile([P, NT], F32, tag="rt1",
                                                   bufs=3, name="rt1")
                                nc.vector.tensor_mul(
                                    rop[:], raw[:], cs_t[:, sl])
                                nc.vector.tensor_mul(
                                    tmp[:], rsw[:], sw_t[:, sl])
                                nc.vector.tensor_add(rop[:], rop[:], tmp[:])
                                nc.scalar.dma_start(
                                    qk_spill[feat * P:(feat + 1) * P, sl],
                                    rop[:],
                                )

            # ---------------- phase C: SDPA (j-outer) ----------------
            with tc.tile_pool(name="y_res", bufs=1) as y_res:
                y_tiles = [
                    y_res.tile([P, T], F32R, tag=f"y{h}", name=f"y{h}")
                    for h in range(HPC)
                ]
                with tc.tile_pool(name="wp_pool", bufs=1) as wp_pool:
                  wp_t = [
                      wp_pool.tile([P, C], F32R, tag=f"wp{hh}",
                                   name=f"wp{hh}")
                      for hh in range(HPC)
                  ]
                  with (
                    tc.tile_pool(name="qk_pool", bufs=1) as qk_pool,
                    tc.tile_pool(name="sd_pool", bufs=1) as sd_pool,
                    tc.tile_pool(name="psum_c", bufs=1, space="PSUM") as psum_c,
                  ):
                    tri_t = qk_pool.tile([P, P], F32, tag="tri", name="tri")
                    nc.sync.dma_start(tri_t[:], tri[:])

                    for h in range(HPC):
                        # spread projection-weight prefetch across heads
                        nc.scalar.dma_start(
                            wp_t[h][:], wp[h * P:(h + 1) * P, :])
                        qh = qk_pool.tile([P, T], F32R, tag="qh", bufs=2,
                                          name="qh")
                        nc.sync.dma_start(qh[:], qk_spill[h * P:(h + 1) * P, :])
                        kh = qk_pool.tile([P, T], F32R, tag="kh", bufs=2,
                                          name="kh")
                        nc.sync.dma_start(
                            kh[:], qk_spill[(HPC + h) * P:(HPC + h + 1) * P, :]
                        )
                        # v for this head: one DMA, [128, 16*128] (j, d)
                        vh3 = qk_pool.tile([P, T // P, P], F32R, tag="vh",
                                           bufs=2, name="vh3")
                        nc.sync.dma_start(
                            vh3[:],
                            v_spill[:, h * D:(h + 1) * D].rearrange(
                                "(j p) d -> p j d", p=P
                            ),
                        )
                        psy = [
                            psum_c.tile([P, NT], F32, tag="psy", bufs=5,
                                        name="psy")
                            for t in range(TT)
                        ]
                        p_sum = [
                            sd_pool.tile([P, NT], F32R, tag=f"p_sum{t}",
                                         bufs=2, name=f"p_sum{t}")
                            for t in range(TT)
                        ]
                        for j in range(T // P):
                            jsl = slice(j * P, (j + 1) * P)
                            for t in range(j // 4, TT):
                                diag = (t == j // 4)
                                off = (j % 4) * P if diag else 0
                                qsl = slice(t * NT + off, (t + 1) * NT)
                                pss = psum_c.tile([P, NT], F32, tag="pss",
                                                  bufs=3, name="pss")
                                _mm(nc, pss[:, off:], kh[:, jsl], qh[:, qsl],
                                    start=True, stop=True)
                                p = sd_pool.tile([P, NT], F32R, tag="p",
                                                 bufs=4, name="p")
                                nc.scalar.activation(
                                    p[:, off:], pss[:, off:], AF.Exp)
                                if diag:
                                    nc.vector.tensor_mul(
                                        p[:, off:off + P],
                                        p[:, off:off + P],
                                        tri_t[:],
                                    )
                                if j == 0:
                                    nc.vector.tensor_copy(p_sum[t][:], p[:])
                                else:
                                    nc.vector.tensor_add(
                                        p_sum[t][:, off:], p_sum[t][:, off:],
                                        p[:, off:],
                                    )
                                _mm(nc, psy[t][:, off:], vh3[:, j, :],
                                    p[:, off:],
                                    start=(j == 0), stop=(j == 4 * t + 3))
                        for t in range(TT):
                            psd = psum_c.tile([P, NT], F32, tag="pss", bufs=3,
                                              name="psd")
                            _mm(nc, psd[:], ones_t[:], p_sum[t][:],
                                start=True, stop=True)
                            rden = sd_pool.tile([P, NT], F32, tag="rden",
                                                bufs=2, name="rden")
                            nc.vector.reciprocal(rden[:], psd[:])
                            nc.vector.tensor_mul(
                                y_tiles[h][:, t * NT:(t + 1) * NT],
                                psy[t][:], rden[:],
                            )

                  # ---------------- phase D: projection ----------------
                  with (
                      tc.tile_pool(name="ot_pool", bufs=1) as ot_pool,
                      tc.tile_pool(name="psum_o", bufs=1,
                                   space="PSUM") as psum_o,
                  ):
                      for m in range(T // P):
                          msl = slice(m * P, (m + 1) * P)
                          pso = [
                              psum_o.tile([P, NT], F32, tag=f"pso{n}",
                                          bufs=2, name=f"pso{n}")
                              for n in range(4)
                          ]
                          for hh in range(HPC):
                              lhsT = y_tiles[hh][:, msl]
                              for n in range(4):
                                  _mm(nc, pso[n][:], lhsT,
                                      wp_t[hh][:, n * NT:(n + 1) * NT],
                                      start=(hh == 0), stop=(hh == HPC - 1))
                          ot = ot_pool.tile([P, C], F32, tag="ot", bufs=3,
                                            name="ot")
                          for n in range(4):
                              nc.scalar.copy(
                                  ot[:, n * NT:(n + 1) * NT], pso[n][:])
                          nc.scalar.dma_start(out[msl, :], ot[:])

    nc.finalize()
    return nc


def prep_inputs(x, w_attn, b_attn, w_proj, b_proj):
    """Build the 8 per-core input maps from full inputs."""
    x = np.asarray(x, dtype=np.float32)
    w_attn = np.asarray(w_attn, dtype=np.float32)
    b_attn = np.asarray(b_attn, dtype=np.float32)
    w_proj = np.asarray(w_proj, dtype=np.float32)

    scale = np.float32(1.0 / np.sqrt(D))

    inv_freq = 1.0 / (ROPE_BASE ** (np.arange(0, D, 2, dtype=np.float32) / D))
    tpos = np.arange(T, dtype=np.float32)
    ang = np.outer(tpos, inv_freq)  # [T, 64]
    cos_t, sin_t = np.cos(ang).T, np.sin(ang).T  # [64, T]
    cs = np.ascontiguousarray(
        np.concatenate([cos_t, cos_t], axis=0)).astype(np.float32)
    sw = np.ascontiguousarray(
        np.concatenate([-sin_t, sin_t], axis=0)).astype(np.float32)

    qq = np.arange(P)
    kk = np.arange(P)[:, None]
    tri = np.ascontiguousarray(
        (qq[None, :] >= kk).astype(np.float32))  # [128,128] causal triangle

    onesm = np.ones((P, P), dtype=np.float32)

    in_maps = []
    for core in range(8):
        b = core // 2
        hg = core % 2
        heads = range(hg * HPC, (hg + 1) * HPC)
        qcols = np.concatenate([np.arange(h * D, (h + 1) * D) for h in heads])
        kcols = qcols + C
        vcols = qcols + 2 * C

        wq = w_attn[:, qcols] * scale
        wk = w_attn[:, kcols]
        wqk_s = np.ascontiguousarray(np.concatenate([wq, wk], axis=1))
        bqk_s = np.concatenate([b_attn[qcols] * scale, b_attn[kcols]])
        bqk_s = np.ascontiguousarray(bqk_s.reshape(QK_CHUNKS, P).T)
        wv_s = np.ascontiguousarray(w_attn[:, vcols])
        bv_s = np.ascontiguousarray(
            np.broadcast_to(b_attn[vcols][None, :], (P, HPC * D)))
        wp_s = np.ascontiguousarray(w_proj[qcols, :])
        xt_s = np.ascontiguousarray(x[b].T)

        in_maps.append({
            "xt": xt_s, "wqk": wqk_s, "bqk": bqk_s, "wv": wv_s, "bvrep": bv_s,
            "cs": cs, "sw": sw, "tri": tri, "onesm": onesm, "wp": wp_s,
        })
    return in_maps


def _get_program():
    if "nc" not in _CACHE:
        _CACHE["nc"] = build_program()
    return _CACHE["nc"]


def _postprocess(outs, b_proj):
    b_proj = np.asarray(b_proj, dtype=np.float32)
    return np.stack(
        [outs[2 * b] + outs[2 * b + 1] + b_proj[None, :] for b in range(B)]
    ).astype(np.float32)


def _run(inputs, trace=False):
    from concourse.bass_utils import run_bass_kernel_spmd

    nc = _get_program()
    in_maps = prep_inputs(
        inputs["x"], inputs["w_attn"], inputs["b_attn"],
        inputs["w_proj"], inputs["b_proj"],
    )
    res = run_bass_kernel_spmd(nc, in_maps, core_ids=list(range(8)),
                               trace=trace)
    full = _postprocess([r["out"] for r in res.results], inputs["b_proj"])
    return full, res


def kernel(**inputs):
    full, _ = _run(inputs, trace=False)
    return full


if __name__ == "__main__":
    _get_program()
    print("built ok")
